# revision 72
# baseline (speedup 1.0000x reference)
"""Trainium2 Bass kernel for the per-task embedding MLP (embedding_lookup).

Computation (per sample j with task t = task_ids[j]):
    h      = x[j] @ l1_emb[t].reshape(256, 128) + l1_bias[t]
    g      = gelu_exact(h)
    out[j] = sum(g * l2_emb[t]) + l2_bias[t, 0]

Strategy: shard the *task* axis across the 8 cores (125 contiguous tasks per
core), so each core streams a contiguous slab of l1_emb exactly once (the
memory roofline), instead of gathering a 128 KiB row per sample (4x more
traffic).  Samples are routed (host-side index math only) to the core owning
their task and packed into a fixed slot grid of W=16 columns per group
(tasks with more than W samples get extra groups with duplicated weight
rows), so all 8 cores run one identical SPMD program: per group, two K=128
matmuls of the task's [256,128] weights against its [256,W] x-columns
accumulate hT[128, NG*W] in PSUM; the epilogue does bias-add + gelu (ACT
table) + w2-mult on column-broadcast views and reduces over hidden via a
ones-vector matmul.

The stage-1 matmul operands (x, w1) are cast to fp16 on the host: fp32
matmuls on trn2 lower to LOW/HIGH double passes (~460 ns/task measured vs
~150 ns for fp16) and fp16 also halves the dominant l1_emb DMA traffic.
Accumulation (PSUM) and the whole epilogue stay fp32; measured end-to-end
L2 relative error ~3e-4.
"""

import numpy as np

import concourse.bacc as bacc
import concourse.mybir as mybir
import concourse.tile as tile
from concourse.bass_utils import run_bass_kernel_spmd

NUM_TASKS = 1000
N_FEATURES = 256
HIDDEN = 128
BATCH = 4096
N_CORES = 8
TPC = NUM_TASKS // N_CORES  # tasks per core = 125
GRP = 5                     # tasks per w1 DMA

INV_SQRT2 = float(1.0 / np.sqrt(2.0))

# Module-level knobs for the test harness (the grader just calls kernel()).
MM_DTYPE = "float16"  # "float16" (fast path) or "float32" (exact fallback)
EPILOGUE = "gelu"  # "gelu" (ACT Gelu table) or "erf" (0.5x(1+erf(x/sqrt2)))
SPLIT_LDW = False  # split weight loads into two 64-col halves (col-groups)
TRACE = False
TMPDIR = None  # optional fixed artifact dir for profiling runs
SIM_CORES = None  # e.g. [0]: run CoreSim for those cores instead of hardware
SIM_EXECUTOR_CLS = None  # optional InstructionExecutor subclass for CoreSim
LAST_RESULTS = None

_PROGRAM_CACHE = {}


def _block_sizes(W, NG):
    """Group counts per PSUM block.  Sizes ramp up (tiny first block so the
    first w1 DMA's completion semaphore fires early) and down (tiny last
    blocks so the epilogue chain after the final matmul is short)."""
    assert 512 % W == 0
    GB = (512 // W // GRP) * GRP  # groups per PSUM block (GB*W <= 512, GRP|GB)
    assert GB >= GRP
    head = [GRP, 2 * GRP]
    tail = [2 * GRP, GRP, GRP]
    rem = NG - sum(head) - sum(tail)
    assert rem > 0
    sizes = head + [GB] * (rem // GB) + ([rem % GB] if rem % GB else []) + tail
    assert sum(sizes) == NG and all(s <= GB and s % GRP == 0 for s in sizes)
    return sizes


def _build_program(W, NG, mm_dtype, epilogue):
    """Emit the SPMD Tile program for slot width W and NG groups per core."""
    sizes = _block_sizes(W, NG)
    use_gelu = epilogue == "gelu"
    NSLOT = NG * W
    f32 = mybir.dt.float32
    mdt = getattr(mybir.dt, mm_dtype)

    nc = bacc.Bacc("TRN2", target_bir_lowering=False, debug=False)

    xT_d = nc.dram_tensor("xT", [2, 128, NSLOT], mdt, kind="ExternalInput").ap()
    # w1 slab, host-packed per block in partition-major [128, gbt, 2, 128]
    # layout, one contiguous region per block (chunked DMAs each)
    w1_d = nc.dram_tensor(
        "w1s", [NG * N_FEATURES * HIDDEN], mdt, kind="ExternalInput"
    ).ap()
    b1_d = nc.dram_tensor("b1Ts", [128, NG], f32, kind="ExternalInput").ap()
    w2_d = nc.dram_tensor("w2T", [128, NG], f32, kind="ExternalInput").ap()
    b2_d = nc.dram_tensor("b2r", [1, NG], f32, kind="ExternalInput").ap()
    out_d = nc.dram_tensor("out", [1, NSLOT], f32, kind="ExternalOutput").ap()

    act_fn = (
        mybir.ActivationFunctionType.Gelu
        if use_gelu
        else mybir.ActivationFunctionType.Erf
    )
    add = mybir.AluOpType.add
    mult = mybir.AluOpType.mult

    with tile.TileContext(nc) as tc:
        with (
            tc.tile_pool(name="const", bufs=1) as constp,
            tc.tile_pool(name="w1pool", bufs=4) as w1p,
            tc.tile_pool(name="work", bufs=3) as workp,
            tc.tile_pool(name="hpsum", bufs=5, space="PSUM") as hpsp,
            tc.tile_pool(name="opsum", bufs=2, space="PSUM") as opsp,
        ):
            # x columns, transposed, as two K-chunks of [128, NSLOT].
            # Non-w1 traffic goes through SWDGE (gpsimd) so the sync HWDGE
            # ring carries nothing but the dominant w1 stream (HWDGE DMAs
            # execute FIFO per issuing engine; anything sharing the ring
            # would delay it).
            xc0 = constp.tile([128, NSLOT], mdt)
            xc1 = constp.tile([128, NSLOT], mdt)
            # x slices for the two small lead-in blocks first, then the rest,
            # so block 0's matmuls aren't gated behind the full 1 MB transfer
            c0 = sizes[0] * W
            c1 = (sizes[0] + sizes[1]) * W
            for lo, hi in ((0, c0), (c0, c1), (c1, NSLOT)):
                nc.gpsimd.dma_start(out=xc0[:, lo:hi], in_=xT_d[0][:, lo:hi])
                nc.gpsimd.dma_start(out=xc1[:, lo:hi], in_=xT_d[1][:, lo:hi])

            # fp16 ones-vector + fp16 product: the hidden-dim reduce matmul
            # runs single-pass (fp32 would take the ~1 us LOW/HIGH path and
            # stall the PE queue between blocks); accumulation stays fp32
            cones = constp.tile([128, 1], mdt)
            nc.vector.memset(cones, 1.0 if use_gelu else INV_SQRT2)

            out_sb = constp.tile([1, NSLOT], f32)

            b1T = w2T = b2r = None
            w1off = 0
            for b, gbt in enumerate(sizes):
                g0 = sum(sizes[:b])
                cols = gbt * W
                base = g0 * W
                csl = slice(base, base + cols)

                ps = hpsp.tile([128, cols], mybir.dt.float32, tag="hps")
                ln = 128 * gbt * 2 * 128
                w1t = w1p.tile([128, gbt, 2, 128], mdt, tag="w1t")
                blk = w1_d[w1off:w1off + ln].rearrange(
                    "(p g c h) -> p g c h", p=128, g=gbt, c=2
                )
                # Sub-DMAs with fine-grained completion sems (subtile deps
                # let matmuls start as each chunk lands).  Block 0 uses
                # single-task chunks so the very first matmul isn't gated
                # behind a full GRP-task transfer.  (Alternating chunks
                # across the sync+scalar HWDGE rings was tried and is NOT
                # faster — the single ring isn't issue/bubble-bound.)
                step = 1 if b == 0 else GRP
                for q in range(gbt // step):
                    qs = slice(q * step, (q + 1) * step)
                    nc.sync.dma_start(out=w1t[:, qs], in_=blk[:, qs])
                w1off += ln
                if b == 0:
                    # consts ride SWDGE after the first w1 block; they're
                    # not needed until the first epilogue
                    b1T = constp.tile([128, NG], f32)
                    nc.gpsimd.dma_start(out=b1T, in_=b1_d)
                    w2T = constp.tile([128, NG], f32)
                    nc.gpsimd.dma_start(out=w2T, in_=w2_d)
                    b2r = constp.tile([1, NG], f32)
                    nc.gpsimd.dma_start(out=b2r, in_=b2_d)
                for jj in range(gbt):
                    sl = slice(jj * W, (jj + 1) * W)
                    xsl = slice(base + jj * W, base + (jj + 1) * W)
                    if SPLIT_LDW:
                        # two M=64 col-group halves: halves the per-LDWEIGHTS
                        # column count; the PE can stream the two loads via
                        # separate XBUS lanes
                        for hf in (0, 1):
                            hp = slice(64 * hf, 64 * (hf + 1))
                            nc.tensor.matmul(
                                ps[hp, sl], lhsT=w1t[:, jj, 0][:, hp],
                                rhs=xc0[:, xsl], start=True, stop=False,
                            )
                            nc.tensor.matmul(
                                ps[hp, sl], lhsT=w1t[:, jj, 1][:, hp],
                                rhs=xc1[:, xsl], start=False, stop=True,
                            )
                    else:
                        nc.tensor.matmul(
                            ps[:, sl], lhsT=w1t[:, jj, 0], rhs=xc0[:, xsl],
                            start=True, stop=False,
                        )
                        nc.tensor.matmul(
                            ps[:, sl], lhsT=w1t[:, jj, 1], rhs=xc1[:, xsl],
                            start=False, stop=True,
                        )

                # hs = h * s + b1*s  (s = 1/sqrt(2) for the erf path, 1 for
                # gelu; b1Ts is host-scaled by s).  Ops run on half-block
                # slices so the ACT/DVE chain pipelines within a block and
                # the after-last-matmul tail is shorter.
                hs = workp.tile([128, cols], f32, tag="hs")
                esb = workp.tile([128, cols], f32, tag="esb")
                prodt = workp.tile([128, cols], mdt, tag="prodt")
                halves = [(0, gbt // 2), (gbt // 2, gbt)] if gbt > GRP else [(0, gbt)]
                for ga, gz in halves:
                    hsl = slice(ga * W, gz * W)
                    n_g = gz - ga
                    b1v = (
                        b1T[:, g0 + ga:g0 + gz]
                        .unsqueeze(2).broadcast_to([128, n_g, W])
                    )
                    nc.vector.scalar_tensor_tensor(
                        hs[:, hsl].rearrange("p (g w) -> p g w", w=W),
                        ps[:, hsl].rearrange("p (g w) -> p g w", w=W),
                        1.0 if use_gelu else INV_SQRT2, b1v, op0=mult, op1=add,
                    )
                    nc.scalar.activation(esb[:, hsl], hs[:, hsl], act_fn)
                    if not use_gelu:
                        # tt = (e + 1) * hs = sqrt(2) * gelu(h)  (in-place)
                        nc.vector.scalar_tensor_tensor(
                            esb[:, hsl], esb[:, hsl], 1.0, hs[:, hsl],
                            op0=add, op1=mult,
                        )
                    # prod = g * w2 (column-broadcast), cast to fp16
                    w2v = (
                        w2T[:, g0 + ga:g0 + gz]
                        .unsqueeze(2).broadcast_to([128, n_g, W])
                    )
                    nc.vector.tensor_mul(
                        prodt[:, hsl].rearrange("p (g w) -> p g w", w=W),
                        esb[:, hsl].rearrange("p (g w) -> p g w", w=W),
                        w2v,
                    )
                # reduce over hidden: [1, cols] = cones.T @ prod
                ops = opsp.tile([1, cols], mybir.dt.float32, tag="ops")
                nc.tensor.matmul(ops, lhsT=cones, rhs=prodt, start=True, stop=True)
                # + b2 (column-broadcast), into the output staging tile
                b2v = b2r[:, g0:g0 + gbt].unsqueeze(2).broadcast_to([1, gbt, W])
                nc.vector.tensor_add(
                    out_sb[:, csl].rearrange("p (g w) -> p g w", w=W),
                    ops.rearrange("p (g w) -> p g w", w=W),
                    b2v,
                )
                # tail blocks' outputs are merged into one DMA on the sync
                # ring after the loop (idle ring, faster HWDGE completion,
                # single issue instead of three serialized ones)
                if b < len(sizes) - 3:
                    nc.gpsimd.dma_start(out=out_d[:, csl], in_=out_sb[:, csl])

            tb = sum(sizes[:-3]) * W
            nc.sync.dma_start(out=out_d[:, tb:], in_=out_sb[:, tb:])

    nc.compile()
    return nc


def _get_program(W, NG, mm_dtype, epilogue):
    key = (W, NG, mm_dtype, epilogue)
    if key not in _PROGRAM_CACHE:
        _PROGRAM_CACHE[key] = _build_program(W, NG, mm_dtype, epilogue)
    return _PROGRAM_CACHE[key]


def kernel(x, task_ids, l1_emb, l1_bias, l2_emb, l2_bias):
    global LAST_RESULTS
    x = np.ascontiguousarray(np.asarray(x, dtype=np.float32))
    tid = np.asarray(task_ids).astype(np.int64)
    l1_emb = np.ascontiguousarray(np.asarray(l1_emb, dtype=np.float32))
    l1_bias = np.ascontiguousarray(np.asarray(l1_bias, dtype=np.float32))
    l2_emb = np.ascontiguousarray(np.asarray(l2_emb, dtype=np.float32))
    l2_bias = np.ascontiguousarray(np.asarray(l2_bias, dtype=np.float32))

    B = x.shape[0]
    assert x.shape == (BATCH, N_FEATURES) and tid.shape == (BATCH,)

    mdt = np.float16 if MM_DTYPE == "float16" else np.float32
    W = 16

    # A "group" is (task, slice of up to W of its samples).  Tasks with more
    # than W samples get several groups (their w1 row is duplicated in the
    # slab); tasks with no samples still get one group so that in the common
    # case the slab is exactly the core's contiguous l1_emb slice.
    counts = np.bincount(tid, minlength=NUM_TASKS)
    ngroups = np.maximum(1, -(-counts // W)).astype(np.int64)  # per task
    ng_core = ngroups.reshape(N_CORES, TPC).sum(axis=1)
    NG = -(-int(ng_core.max()) // GRP) * GRP  # round up to a GRP multiple
    NSLOT = NG * W

    # within-core group base of each task
    gbase = np.empty(NUM_TASKS, dtype=np.int64)
    for c in range(N_CORES):
        sl = slice(c * TPC, (c + 1) * TPC)
        cs = np.cumsum(ngroups[sl])
        gbase[sl] = cs - ngroups[sl]

    # slot routing: sample j -> (core, slot)
    order = np.argsort(tid, kind="stable")
    sorted_tid = tid[order]
    starts = np.flatnonzero(np.r_[True, np.diff(sorted_tid) != 0])
    run_len = np.diff(np.r_[starts, B])
    run_pos = np.arange(B) - np.repeat(starts, run_len)
    occ = np.empty(B, dtype=np.int64)
    occ[order] = run_pos
    core = tid // TPC
    slot = (gbase[tid] + occ // W) * W + occ % W

    # scatter x into per-core transposed, padded slot grids
    xT = np.zeros((N_CORES, N_FEATURES, NSLOT), dtype=mdt)
    xT[core, :, slot] = x.astype(mdt)

    inv = np.float32(INV_SQRT2)
    sizes = _block_sizes(W, NG)
    in_maps = []
    for c in range(N_CORES):
        t0 = c * TPC
        sl = slice(t0, t0 + TPC)
        # task id of each group (padded to NG with the core's first task)
        gtask = np.repeat(np.arange(t0, t0 + TPC), ngroups[sl])
        if len(gtask) < NG:
            gtask = np.r_[gtask, np.full(NG - len(gtask), t0)]
        rows = l1_emb[gtask]  # [NG, 32768]
        # pack w1 per block: [gbt, 2, 128, 128] -> [128, gbt, 2, 128] flat
        parts = []
        cum = 0
        for gbt in sizes:
            blk = rows[cum:cum + gbt]
            blk = blk.reshape(gbt, 2, 128, 128).transpose(2, 0, 1, 3)
            parts.append(blk.astype(mdt).reshape(-1))
            cum += gbt
        in_maps.append({
            "xT": np.ascontiguousarray(xT[c].reshape(2, 128, NSLOT)),
            "w1s": np.concatenate(parts),
            "b1Ts": np.ascontiguousarray(l1_bias[gtask].T)
            * (np.float32(1.0) if EPILOGUE == "gelu" else inv),
            "w2T": np.ascontiguousarray(l2_emb[gtask].T),
            "b2r": np.ascontiguousarray(l2_bias[gtask].reshape(1, NG)),
        })

    nc = _get_program(W, NG, MM_DTYPE, EPILOGUE)
    if SIM_CORES is not None:
        from concourse.bass_interp import CoreSim

        sim_results = []
        for c in range(N_CORES):
            if c in SIM_CORES:
                kw = {}
                if SIM_EXECUTOR_CLS is not None:
                    kw["executor_cls"] = SIM_EXECUTOR_CLS
                sim = CoreSim(nc, publish_trace=False, **kw)
                for k, v in in_maps[c].items():
                    sim.tensor(k)[:] = v
                sim.simulate()
                sim_results.append({"out": np.array(sim.tensor("out"))})
            else:
                sim_results.append({"out": np.zeros((1, NSLOT), np.float32)})
        outs = np.stack([r["out"].reshape(NSLOT) for r in sim_results])
        logits = outs[core, slot]
        return logits[:, None].astype(np.float32)

    res = run_bass_kernel_spmd(
        nc, in_maps, core_ids=list(range(N_CORES)), trace=TRACE, tmpdir=TMPDIR,
    )
    LAST_RESULTS = res

    outs = np.stack([r["out"].reshape(NSLOT) for r in res.results])
    logits = outs[core, slot]
    return logits[:, None].astype(np.float32)


# revision 73
# speedup vs baseline: 1.0292x; 1.0292x over previous
"""Trainium2 Bass kernel for the per-task embedding MLP (embedding_lookup).

Computation (per sample j with task t = task_ids[j]):
    h      = x[j] @ l1_emb[t].reshape(256, 128) + l1_bias[t]
    g      = gelu_exact(h)
    out[j] = sum(g * l2_emb[t]) + l2_bias[t, 0]

Strategy: shard the *task* axis across the 8 cores (125 contiguous tasks per
core), so each core streams a contiguous slab of l1_emb exactly once (the
memory roofline), instead of gathering a 128 KiB row per sample (4x more
traffic).  Samples are routed (host-side index math only) to the core owning
their task and packed into a fixed slot grid of W=16 columns per group
(tasks with more than W samples get extra groups with duplicated weight
rows), so all 8 cores run one identical SPMD program: per group, two K=128
matmuls of the task's [256,128] weights against its [256,W] x-columns
accumulate hT[128, NG*W] in PSUM; the epilogue does bias-add + gelu (ACT
table) + w2-mult on column-broadcast views and reduces over hidden via a
ones-vector matmul.

The stage-1 matmul operands (x, w1) are cast to fp16 on the host: fp32
matmuls on trn2 lower to LOW/HIGH double passes (~460 ns/task measured vs
~150 ns for fp16) and fp16 also halves the dominant l1_emb DMA traffic.
Accumulation (PSUM) and the whole epilogue stay fp32; measured end-to-end
L2 relative error ~3e-4.
"""

import numpy as np

import concourse.bacc as bacc
import concourse.mybir as mybir
import concourse.tile as tile
from concourse.bass_utils import run_bass_kernel_spmd

NUM_TASKS = 1000
N_FEATURES = 256
HIDDEN = 128
BATCH = 4096
N_CORES = 8
TPC = NUM_TASKS // N_CORES  # tasks per core = 125
GRP = 5                     # tasks per w1 DMA

INV_SQRT2 = float(1.0 / np.sqrt(2.0))

# Module-level knobs for the test harness (the grader just calls kernel()).
MM_DTYPE = "float16"  # "float16" (fast path) or "float32" (exact fallback)
EPILOGUE = "gelu"  # "gelu" (ACT Gelu table) or "erf" (0.5x(1+erf(x/sqrt2)))
SPLIT_LDW = False  # split weight loads into two 64-col halves (col-groups)
TRACE = False
TMPDIR = None  # optional fixed artifact dir for profiling runs
SIM_CORES = None  # e.g. [0]: run CoreSim for those cores instead of hardware
SIM_EXECUTOR_CLS = None  # optional InstructionExecutor subclass for CoreSim
LAST_RESULTS = None

_PROGRAM_CACHE = {}


def _block_sizes(W, NG):
    """Group counts per PSUM block.  Sizes ramp up (tiny first block so the
    first w1 DMA's completion semaphore fires early) and down (tiny last
    blocks so the epilogue chain after the final matmul is short)."""
    assert 512 % W == 0
    GB = (512 // W // GRP) * GRP  # groups per PSUM block (GB*W <= 512, GRP|GB)
    assert GB >= GRP
    head = [GRP, 2 * GRP]
    tail = [2 * GRP, GRP, GRP]
    rem = NG - sum(head) - sum(tail)
    assert rem > 0
    sizes = head + [GB] * (rem // GB) + ([rem % GB] if rem % GB else []) + tail
    assert sum(sizes) == NG and all(s <= GB and s % GRP == 0 for s in sizes)
    return sizes


def _build_program(W, NG, mm_dtype, epilogue):
    """Emit the SPMD Tile program for slot width W and NG groups per core."""
    sizes = _block_sizes(W, NG)
    use_gelu = epilogue == "gelu"
    NSLOT = NG * W
    f32 = mybir.dt.float32
    mdt = getattr(mybir.dt, mm_dtype)

    nc = bacc.Bacc("TRN2", target_bir_lowering=False, debug=False)

    xT_d = nc.dram_tensor("xT", [2, 128, NSLOT], mdt, kind="ExternalInput").ap()
    # w1 slab, host-packed per block in partition-major [128, gbt, 2, 128]
    # layout, one contiguous region per block (chunked DMAs each)
    w1_d = nc.dram_tensor(
        "w1s", [NG * N_FEATURES * HIDDEN], mdt, kind="ExternalInput"
    ).ap()
    b1_d = nc.dram_tensor("b1Ts", [128, NG], f32, kind="ExternalInput").ap()
    w2_d = nc.dram_tensor("w2T", [128, NG], f32, kind="ExternalInput").ap()
    b2_d = nc.dram_tensor("b2r", [1, NG], f32, kind="ExternalInput").ap()
    out_d = nc.dram_tensor("out", [1, NSLOT], f32, kind="ExternalOutput").ap()

    act_fn = (
        mybir.ActivationFunctionType.Gelu
        if use_gelu
        else mybir.ActivationFunctionType.Erf
    )
    add = mybir.AluOpType.add
    mult = mybir.AluOpType.mult

    with tile.TileContext(nc) as tc:
        with (
            tc.tile_pool(name="const", bufs=1) as constp,
            tc.tile_pool(name="w1pool", bufs=4) as w1p,
            tc.tile_pool(name="work", bufs=3) as workp,
            tc.tile_pool(name="hpsum", bufs=5, space="PSUM") as hpsp,
            tc.tile_pool(name="opsum", bufs=2, space="PSUM") as opsp,
        ):
            # x columns, transposed, as two K-chunks of [128, NSLOT].
            # Non-w1 traffic goes through SWDGE (gpsimd) so the sync HWDGE
            # ring carries nothing but the dominant w1 stream (HWDGE DMAs
            # execute FIFO per issuing engine; anything sharing the ring
            # would delay it).
            xc0 = constp.tile([128, NSLOT], mdt)
            xc1 = constp.tile([128, NSLOT], mdt)
            # x slices for the two small lead-in blocks first, then the rest,
            # so block 0's matmuls aren't gated behind the full 1 MB transfer
            c0 = sizes[0] * W
            c1 = (sizes[0] + sizes[1]) * W
            for lo, hi in ((0, c0), (c0, c1), (c1, NSLOT)):
                nc.gpsimd.dma_start(out=xc0[:, lo:hi], in_=xT_d[0][:, lo:hi])
                nc.gpsimd.dma_start(out=xc1[:, lo:hi], in_=xT_d[1][:, lo:hi])

            # fp16 ones-vector + fp16 product: the hidden-dim reduce matmul
            # runs single-pass (fp32 would take the ~1 us LOW/HIGH path and
            # stall the PE queue between blocks); accumulation stays fp32
            cones = constp.tile([128, 1], mdt)
            nc.vector.memset(cones, 1.0 if use_gelu else INV_SQRT2)

            out_sb = constp.tile([1, NSLOT], f32)

            b1T = w2T = b2r = None
            w1off = 0
            for b, gbt in enumerate(sizes):
                g0 = sum(sizes[:b])
                cols = gbt * W
                base = g0 * W
                csl = slice(base, base + cols)

                ps = hpsp.tile([128, cols], mybir.dt.float32, tag="hps")
                ln = 128 * gbt * 2 * 128
                w1t = w1p.tile([128, gbt, 2, 128], mdt, tag="w1t")
                blk = w1_d[w1off:w1off + ln].rearrange(
                    "(p g c h) -> p g c h", p=128, g=gbt, c=2
                )
                # Sub-DMAs with fine-grained completion sems (subtile deps
                # let matmuls start as each chunk lands).  Block 0 uses
                # single-task chunks so the very first matmul isn't gated
                # behind a full GRP-task transfer.  (Alternating chunks
                # across the sync+scalar HWDGE rings was tried and is NOT
                # faster — the single ring isn't issue/bubble-bound.)
                # full blocks use 2*GRP-task chunks (5 KB descriptors,
                # half the packet overhead and issue count)
                if b == 0:
                    step = 1
                elif gbt % (2 * GRP) == 0:
                    step = 2 * GRP
                else:
                    step = GRP
                for q in range(gbt // step):
                    qs = slice(q * step, (q + 1) * step)
                    nc.sync.dma_start(out=w1t[:, qs], in_=blk[:, qs])
                w1off += ln
                if b == 0:
                    # consts ride SWDGE after the first w1 block; they're
                    # not needed until the first epilogue
                    b1T = constp.tile([128, NG], f32)
                    nc.gpsimd.dma_start(out=b1T, in_=b1_d)
                    w2T = constp.tile([128, NG], f32)
                    nc.gpsimd.dma_start(out=w2T, in_=w2_d)
                    b2r = constp.tile([1, NG], f32)
                    nc.gpsimd.dma_start(out=b2r, in_=b2_d)
                for jj in range(gbt):
                    sl = slice(jj * W, (jj + 1) * W)
                    xsl = slice(base + jj * W, base + (jj + 1) * W)
                    if SPLIT_LDW:
                        # two M=64 col-group halves: halves the per-LDWEIGHTS
                        # column count; the PE can stream the two loads via
                        # separate XBUS lanes
                        for hf in (0, 1):
                            hp = slice(64 * hf, 64 * (hf + 1))
                            nc.tensor.matmul(
                                ps[hp, sl], lhsT=w1t[:, jj, 0][:, hp],
                                rhs=xc0[:, xsl], start=True, stop=False,
                            )
                            nc.tensor.matmul(
                                ps[hp, sl], lhsT=w1t[:, jj, 1][:, hp],
                                rhs=xc1[:, xsl], start=False, stop=True,
                            )
                    else:
                        nc.tensor.matmul(
                            ps[:, sl], lhsT=w1t[:, jj, 0], rhs=xc0[:, xsl],
                            start=True, stop=False,
                        )
                        nc.tensor.matmul(
                            ps[:, sl], lhsT=w1t[:, jj, 1], rhs=xc1[:, xsl],
                            start=False, stop=True,
                        )

                # hs = h * s + b1*s  (s = 1/sqrt(2) for the erf path, 1 for
                # gelu; b1Ts is host-scaled by s).  Ops run on half-block
                # slices so the ACT/DVE chain pipelines within a block and
                # the after-last-matmul tail is shorter.
                hs = workp.tile([128, cols], f32, tag="hs")
                esb = workp.tile([128, cols], f32, tag="esb")
                prodt = workp.tile([128, cols], mdt, tag="prodt")
                halves = [(0, gbt // 2), (gbt // 2, gbt)] if gbt > GRP else [(0, gbt)]
                for ga, gz in halves:
                    hsl = slice(ga * W, gz * W)
                    n_g = gz - ga
                    b1v = (
                        b1T[:, g0 + ga:g0 + gz]
                        .unsqueeze(2).broadcast_to([128, n_g, W])
                    )
                    nc.vector.scalar_tensor_tensor(
                        hs[:, hsl].rearrange("p (g w) -> p g w", w=W),
                        ps[:, hsl].rearrange("p (g w) -> p g w", w=W),
                        1.0 if use_gelu else INV_SQRT2, b1v, op0=mult, op1=add,
                    )
                    nc.scalar.activation(esb[:, hsl], hs[:, hsl], act_fn)
                    if not use_gelu:
                        # tt = (e + 1) * hs = sqrt(2) * gelu(h)  (in-place)
                        nc.vector.scalar_tensor_tensor(
                            esb[:, hsl], esb[:, hsl], 1.0, hs[:, hsl],
                            op0=add, op1=mult,
                        )
                    # prod = g * w2 (column-broadcast), cast to fp16
                    w2v = (
                        w2T[:, g0 + ga:g0 + gz]
                        .unsqueeze(2).broadcast_to([128, n_g, W])
                    )
                    nc.vector.tensor_mul(
                        prodt[:, hsl].rearrange("p (g w) -> p g w", w=W),
                        esb[:, hsl].rearrange("p (g w) -> p g w", w=W),
                        w2v,
                    )
                # reduce over hidden: [1, cols] = cones.T @ prod
                ops = opsp.tile([1, cols], mybir.dt.float32, tag="ops")
                nc.tensor.matmul(ops, lhsT=cones, rhs=prodt, start=True, stop=True)
                # + b2 (column-broadcast), into the output staging tile
                b2v = b2r[:, g0:g0 + gbt].unsqueeze(2).broadcast_to([1, gbt, W])
                nc.vector.tensor_add(
                    out_sb[:, csl].rearrange("p (g w) -> p g w", w=W),
                    ops.rearrange("p (g w) -> p g w", w=W),
                    b2v,
                )
                # tail blocks' outputs are merged into one DMA on the sync
                # ring after the loop (idle ring, faster HWDGE completion,
                # single issue instead of three serialized ones)
                if b < len(sizes) - 3:
                    nc.gpsimd.dma_start(out=out_d[:, csl], in_=out_sb[:, csl])

            tb = sum(sizes[:-3]) * W
            nc.sync.dma_start(out=out_d[:, tb:], in_=out_sb[:, tb:])

    nc.compile()
    return nc


def _get_program(W, NG, mm_dtype, epilogue):
    key = (W, NG, mm_dtype, epilogue)
    if key not in _PROGRAM_CACHE:
        _PROGRAM_CACHE[key] = _build_program(W, NG, mm_dtype, epilogue)
    return _PROGRAM_CACHE[key]


def kernel(x, task_ids, l1_emb, l1_bias, l2_emb, l2_bias):
    global LAST_RESULTS
    x = np.ascontiguousarray(np.asarray(x, dtype=np.float32))
    tid = np.asarray(task_ids).astype(np.int64)
    l1_emb = np.ascontiguousarray(np.asarray(l1_emb, dtype=np.float32))
    l1_bias = np.ascontiguousarray(np.asarray(l1_bias, dtype=np.float32))
    l2_emb = np.ascontiguousarray(np.asarray(l2_emb, dtype=np.float32))
    l2_bias = np.ascontiguousarray(np.asarray(l2_bias, dtype=np.float32))

    B = x.shape[0]
    assert x.shape == (BATCH, N_FEATURES) and tid.shape == (BATCH,)

    mdt = np.float16 if MM_DTYPE == "float16" else np.float32
    W = 16

    # A "group" is (task, slice of up to W of its samples).  Tasks with more
    # than W samples get several groups (their w1 row is duplicated in the
    # slab); tasks with no samples still get one group so that in the common
    # case the slab is exactly the core's contiguous l1_emb slice.
    counts = np.bincount(tid, minlength=NUM_TASKS)
    ngroups = np.maximum(1, -(-counts // W)).astype(np.int64)  # per task
    ng_core = ngroups.reshape(N_CORES, TPC).sum(axis=1)
    NG = -(-int(ng_core.max()) // GRP) * GRP  # round up to a GRP multiple
    NSLOT = NG * W

    # within-core group base of each task
    gbase = np.empty(NUM_TASKS, dtype=np.int64)
    for c in range(N_CORES):
        sl = slice(c * TPC, (c + 1) * TPC)
        cs = np.cumsum(ngroups[sl])
        gbase[sl] = cs - ngroups[sl]

    # slot routing: sample j -> (core, slot)
    order = np.argsort(tid, kind="stable")
    sorted_tid = tid[order]
    starts = np.flatnonzero(np.r_[True, np.diff(sorted_tid) != 0])
    run_len = np.diff(np.r_[starts, B])
    run_pos = np.arange(B) - np.repeat(starts, run_len)
    occ = np.empty(B, dtype=np.int64)
    occ[order] = run_pos
    core = tid // TPC
    slot = (gbase[tid] + occ // W) * W + occ % W

    # scatter x into per-core transposed, padded slot grids
    xT = np.zeros((N_CORES, N_FEATURES, NSLOT), dtype=mdt)
    xT[core, :, slot] = x.astype(mdt)

    inv = np.float32(INV_SQRT2)
    sizes = _block_sizes(W, NG)
    in_maps = []
    for c in range(N_CORES):
        t0 = c * TPC
        sl = slice(t0, t0 + TPC)
        # task id of each group (padded to NG with the core's first task)
        gtask = np.repeat(np.arange(t0, t0 + TPC), ngroups[sl])
        if len(gtask) < NG:
            gtask = np.r_[gtask, np.full(NG - len(gtask), t0)]
        rows = l1_emb[gtask]  # [NG, 32768]
        # pack w1 per block: [gbt, 2, 128, 128] -> [128, gbt, 2, 128] flat
        parts = []
        cum = 0
        for gbt in sizes:
            blk = rows[cum:cum + gbt]
            blk = blk.reshape(gbt, 2, 128, 128).transpose(2, 0, 1, 3)
            parts.append(blk.astype(mdt).reshape(-1))
            cum += gbt
        in_maps.append({
            "xT": np.ascontiguousarray(xT[c].reshape(2, 128, NSLOT)),
            "w1s": np.concatenate(parts),
            "b1Ts": np.ascontiguousarray(l1_bias[gtask].T)
            * (np.float32(1.0) if EPILOGUE == "gelu" else inv),
            "w2T": np.ascontiguousarray(l2_emb[gtask].T),
            "b2r": np.ascontiguousarray(l2_bias[gtask].reshape(1, NG)),
        })

    nc = _get_program(W, NG, MM_DTYPE, EPILOGUE)
    if SIM_CORES is not None:
        from concourse.bass_interp import CoreSim

        sim_results = []
        for c in range(N_CORES):
            if c in SIM_CORES:
                kw = {}
                if SIM_EXECUTOR_CLS is not None:
                    kw["executor_cls"] = SIM_EXECUTOR_CLS
                sim = CoreSim(nc, publish_trace=False, **kw)
                for k, v in in_maps[c].items():
                    sim.tensor(k)[:] = v
                sim.simulate()
                sim_results.append({"out": np.array(sim.tensor("out"))})
            else:
                sim_results.append({"out": np.zeros((1, NSLOT), np.float32)})
        outs = np.stack([r["out"].reshape(NSLOT) for r in sim_results])
        logits = outs[core, slot]
        return logits[:, None].astype(np.float32)

    res = run_bass_kernel_spmd(
        nc, in_maps, core_ids=list(range(N_CORES)), trace=TRACE, tmpdir=TMPDIR,
    )
    LAST_RESULTS = res

    outs = np.stack([r["out"].reshape(NSLOT) for r in res.results])
    logits = outs[core, slot]
    return logits[:, None].astype(np.float32)
